# revision 8
# baseline (speedup 1.0000x reference)
"""Trainium2 Bass kernel for nn_LossFunction_16836271800471 (flatNCE-style loss).

Reference computation (B=4096, M=2, D=1024):
    pos = x[:,0,:]; anc = mean(x[:,1:,:], 1) = x[:,1,:]
    sim[i,j] = cos(pos[i], anc[j])                       # [B,B]
    temploss[j] = logsumexp_{i != j}(sim[i,j] - sim[j,j])
    nloss = mean(exp(temploss - stop_grad(temploss)))    # == 1.0 in fwd
    prec1 = 100 * mean(argmax_j sim[i,j] == i)

nloss is identically 1.0 in the forward pass for any finite input
(exp(t - stop_grad(t)) = exp(0)); only prec1 is data-dependent, and it
needs exactly argmax_j sim[i,j] per row. The device therefore computes
only the row maxes of the similarity matrix:

  - rows sharded 512/core; anchors replicated (no collectives)
  - both operands quantized to fp8 e4m3 on host (normalized rows, all
    |values| < 0.25 so e4m3/e4m3fn encodings agree); matmuls run in
    MatmulPerfMode.DoubleRow (K=256 per instruction, 0.5 cycles/col)
  - anc-block phases: the PE starts once pos + the first 512KB anc
    block have landed (~1MB gate instead of 4.5MB; DMA spread over the
    sync/scalar/gpsimd queues in parallel); weight loads are hidden
    under the previous matmul on TRN2, so per-matmul self-loading is
    free and the loop just walks (col-block, row-block, k)
  - psum [128,512] per (col-block, row-block); 32 DVE max-reduces
    staggered so they overlap the PE stream

Host combines: diag[i] = <posn_i, ancn_i> exactly in fp64; match iff
diag >= rowmax, with rows inside a THRESH=0.012 band (fp8 rowmax error
measured <= 6.5e-3) re-checked exactly in fp64 (vectorized sgemm over
the few suspect rows). nloss = 1.0.
"""

import numpy as np

import concourse.bass as bass
import concourse.tile as tile
from concourse import bacc, mybir
from concourse.bass_utils import run_bass_kernel_spmd

B, M, D = 4096, 2, 1024
NCORES = 8
RB = B // NCORES          # 512 rows per core
P = 128                   # partitions
KT = D // P               # 8 contraction subtiles of 128
KT2 = KT // 2             # 4 DoubleRow k-pairs
MB = RB // P              # 4 row-blocks per core
NBLK = 512                # anc col-block width
NB = B // NBLK            # 8 col-blocks
NHALF = 2                 # anc halves (4 blocks each)
THRESH = 0.012            # fp8 rowmax error bound (measured max 6.5e-3)

F32 = mybir.dt.float32
F8 = mybir.dt.float8e4
AX = mybir.AxisListType
OP = mybir.AluOpType
PM = mybir.MatmulPerfMode.DoubleRow

NP_F8 = mybir.dt.np(F8)

_CACHE = {}


def _build():
    nc = bacc.Bacc("TRN2", target_bir_lowering=False, debug=False,
                   num_devices=NCORES)
    # pos grouped by row-block so the first phase's weights (128 KB)
    # land first: posTI[p, m, k, r128]
    posTI = nc.dram_tensor("posTI", [P, MB, KT * P], F8,
                           kind="ExternalInput").ap()
    ancTI = nc.dram_tensor("ancTI", [P, NB, KT * NBLK], F8,
                           kind="ExternalInput").ap()
    rmf = nc.dram_tensor("rmf", [P, NB * MB], F32,
                         kind="ExternalOutput").ap()

    with tile.TileContext(nc) as tc:
        with (
            tc.tile_pool(name="posp", bufs=1) as posp,
            tc.tile_pool(name="ancp", bufs=NB) as ancp,
            tc.tile_pool(name="outp", bufs=1) as outp,
            tc.tile_pool(name="psmm", bufs=8, space="PSUM") as psmm,
        ):
            # DMA priority order: the two HWDGE queues (sync, scalar) get
            # the gate-critical transfers first (pos row-blocks, anc0);
            # the SWDGE queue (gpsimd, ~3us slower to start) gets blocks
            # that are only needed mid-stream.
            pos_t = posp.tile([P, MB, KT, P], F8)
            nc.sync.dma_start(pos_t[:, 0], posTI[:, 0, :])
            nc.scalar.dma_start(pos_t[:, 1], posTI[:, 1, :])
            nc.sync.dma_start(pos_t[:, 2], posTI[:, 2, :])
            nc.scalar.dma_start(pos_t[:, 3], posTI[:, 3, :])

            anc_ts = []
            for n in range(NB):
                anc_t = ancp.tile([P, KT, NBLK], F8, tag="anc",
                                  name=f"anc{n}")
                anc_ts.append(anc_t)
            half = KT // 2 * NBLK
            nc.sync.dma_start(anc_ts[0][:, 0:KT // 2, :],
                              ancTI[:, 0, 0:half])
            nc.scalar.dma_start(anc_ts[0][:, KT // 2:KT, :],
                                ancTI[:, 0, half:2 * half])
            nc.sync.dma_start(anc_ts[1][:, 0:KT // 2, :],
                              ancTI[:, 1, 0:half])
            nc.scalar.dma_start(anc_ts[1][:, KT // 2:KT, :],
                                ancTI[:, 1, half:2 * half])
            qs = [nc.gpsimd, nc.sync, nc.scalar]
            for n in range(2, NB):
                qs[n % 3].dma_start(anc_ts[n][:], ancTI[:, n, :])

            rm_sb = outp.tile([P, NB * MB], F32)

            for n in range(NB):
                for m in range(MB):
                    ps = psmm.tile([P, NBLK], F32, tag="ps")
                    for k in range(KT2):
                        nc.tensor.matmul(
                            ps[:],
                            pos_t[:, m, 2 * k:2 * k + 2, :],
                            anc_ts[n][:, 2 * k:2 * k + 2, :],
                            start=(k == 0), stop=(k == KT2 - 1),
                            perf_mode=PM)
                    col = n * MB + m
                    nc.vector.tensor_reduce(
                        rm_sb[:, col:col + 1], ps[:], AX.X, OP.max)
            nc.sync.dma_start(rmf[:], rm_sb[:])
    nc.compile()
    return nc


def _get_nc():
    if "nc" not in _CACHE:
        _CACHE["nc"] = _build()
    return _CACHE["nc"]


def _prep(x):
    """Normalize in fp64, quantize to fp8, build per-core SBUF-layouts."""
    x = np.asarray(x, dtype=np.float32)
    assert x.shape == (B, M, D)
    pos = x[:, 0, :]
    anc = x[:, 1:, :].mean(axis=1) if M > 2 else x[:, 1, :]
    pos64 = pos.astype(np.float64)
    anc64 = anc.astype(np.float64)
    posn64 = pos64 / np.linalg.norm(pos64, axis=1, keepdims=True)
    ancn64 = anc64 / np.linalg.norm(anc64, axis=1, keepdims=True)

    pos8 = posn64.astype(np.float32).astype(NP_F8)   # [B, D]
    anc8 = ancn64.astype(np.float32).astype(NP_F8)

    # ancTI[p, n, k*NBLK + c] = ancn.T[k*128+p, n*512+c]
    ancTI = np.ascontiguousarray(
        anc8.T.reshape(KT, P, NB, NBLK).transpose(1, 2, 0, 3)
        .reshape(P, NB, KT * NBLK))
    in_maps = []
    for c in range(NCORES):
        sl = slice(c * RB, (c + 1) * RB)
        # posTI[p, m, k*128 + r] = posn[c*512 + m*128 + r][k*128 + p]
        posTI = np.ascontiguousarray(
            pos8[sl].T.reshape(KT, P, MB, P).transpose(1, 2, 0, 3)
            .reshape(P, MB, KT * P))
        in_maps.append({"posTI": posTI, "ancTI": ancTI})
    return in_maps, posn64, ancn64


def _run_cores(x, trace=False):
    in_maps, posn64, ancn64 = _prep(x)
    nc = _get_nc()
    res = run_bass_kernel_spmd(nc, in_maps, list(range(NCORES)), trace=trace)
    return res, posn64, ancn64


def _assemble(res, posn64, ancn64):
    # device row maxes: row i = 512c + 128m + p -> max over the 2 halves
    rm = np.empty(B, np.float32)
    for c in range(NCORES):
        r = res.results[c]["rmf"].reshape(P, NB, MB)  # [P, n, m]
        for m in range(MB):
            rows = slice(c * RB + m * P, c * RB + (m + 1) * P)
            rm[rows] = r[:, :, m].max(axis=1)

    diag = np.einsum("id,id->i", posn64, ancn64)      # exact fp64

    match = np.zeros(B, dtype=bool)
    suspect = diag >= (rm.astype(np.float64) - THRESH)
    if suspect.any():
        idx = np.where(suspect)[0]
        rows = posn64[idx] @ ancn64.T                 # exact fp64 rows
        match[idx] = rows.argmax(axis=1) == idx
    prec1 = np.float32(match.sum() / B * 100.0)

    # forward-pass flatNCE identity: exp(t - stop_grad(t)) == 1 per column
    nloss = np.float32(1.0)
    return nloss, prec1, rm


def kernel(x):
    res, posn64, ancn64 = _run_cores(x, trace=False)
    nloss, prec1, _ = _assemble(res, posn64, ancn64)
    return nloss, prec1


# revision 26
# speedup vs baseline: 1.1048x; 1.1048x over previous
"""Trainium2 Bass kernel for nn_LossFunction_16836271800471 (flatNCE-style loss).

Reference computation (B=4096, M=2, D=1024):
    pos = x[:,0,:]; anc = mean(x[:,1:,:], 1) = x[:,1,:]
    sim[i,j] = cos(pos[i], anc[j])                       # [B,B]
    temploss[j] = logsumexp_{i != j}(sim[i,j] - sim[j,j])
    nloss = mean(exp(temploss - stop_grad(temploss)))    # == 1.0 in fwd
    prec1 = 100 * mean(argmax_j sim[i,j] == i)

nloss is identically 1.0 in the forward pass for any finite input
(exp(t - stop_grad(t)) = exp(0)); only prec1 is data-dependent, and it
needs exactly argmax_j sim[i,j] per row. The device therefore computes
only the row maxes of the similarity matrix:

  - rows sharded 512/core; anchors replicated (no collectives)
  - both operands quantized to fp8 e4m3 on host (normalized rows, all
    |values| < 0.25 so e4m3/e4m3fn encodings agree); matmuls run in
    MatmulPerfMode.DoubleRow (K=256 per instruction, 2x fp16 rate)
  - anc-block phases: the PE starts once the first anc col-block and
    the first pos row-block chunk have landed; gate-critical blocks
    ride the two HWDGE DMA queues (sync/scalar, ~58-75 GB/s heads,
    live from ~7.2us), pos chunks and mid-stream blocks ride the SWDGE
    queue (more channels but ~1.5us later start + desc prep)
  - PE p-state warmup: throwaway matmuls on a zeroed tile while DMA
    streams in (cold PE runs ~1.8x slower for its first ~3us)
  - psum [128,512] per (col-block, row-block); DVE max-reduces
    staggered so they overlap the PE stream; weight loads are hidden
    under the previous matmul on TRN2, so per-matmul self-loading is
    ~free

Host combines: diag[i] = <posn_i, ancn_i> exactly in fp64; match iff
diag >= rowmax, with rows inside a THRESH=0.012 band (fp8 rowmax error
measured <= 6.5e-3) re-checked exactly in fp64 (vectorized sgemm over
the few suspect rows). nloss = 1.0.
"""

import numpy as np

import concourse.bass as bass
import concourse.tile as tile
from concourse import bacc, mybir
from concourse.bass_utils import run_bass_kernel_spmd

B, M, D = 4096, 2, 1024
NCORES = 8
RB = B // NCORES          # 512 rows per core
P = 128                   # partitions
KT = D // P               # 8 contraction subtiles of 128
KT2 = KT // 2             # 4 DoubleRow k-pairs
MB = RB // P              # 4 row-blocks per core
NBLK = 512                # max anc col-block width
THRESH = 0.012            # fp8 rowmax error bound (measured max 6.5e-3)

F32 = mybir.dt.float32
F8 = mybir.dt.float8e4
AX = mybir.AxisListType
OP = mybir.AluOpType
PM = mybir.MatmulPerfMode.DoubleRow

NP_F8 = mybir.dt.np(F8)

_CACHE = {}

# widths: anc col-block widths (sum = B); hwdge: blocks fed lo/hi over
# the sync/scalar queues in that order; swdge: whole-block DMAs on the
# gpsimd queue (after the pos chunks).
MODES = {
    "v5": dict(widths=[512] * 8, hwdge=[0, 1, 3, 5], swdge=[2, 4, 6, 7]),
    "r4": dict(widths=[256] * 4 + [512] * 6, hwdge=[0, 1, 2, 3, 7],
               swdge=[4, 5, 6, 8, 9]),
}

DMA_MODE = "v5"
WARMUPS = 8


def _widths(dma_mode=None):
    return MODES[DMA_MODE if dma_mode is None else dma_mode]["widths"]


def _build(dma_mode=None, warmups=None):
    mode = MODES[DMA_MODE if dma_mode is None else dma_mode]
    warmups = WARMUPS if warmups is None else warmups
    widths = mode["widths"]
    NBL = len(widths)
    offs = np.concatenate([[0], np.cumsum(widths)])  # col offsets

    nc = bacc.Bacc("TRN2", target_bir_lowering=False, debug=False,
                   num_devices=NCORES)
    # pos grouped by row-block so the first phase's weights (128 KB)
    # land first: posTI[p, m, k*128 + r]
    posTI = nc.dram_tensor("posTI", [P, MB, KT * P], F8,
                           kind="ExternalInput").ap()
    # anc blocks packed consecutively: block b at [KT*offs[b], KT*offs[b+1])
    ancTI = nc.dram_tensor("ancTI", [P, KT * B], F8,
                           kind="ExternalInput").ap()
    rmf = nc.dram_tensor("rmf", [P, NBL * MB], F32,
                         kind="ExternalOutput").ap()

    with tile.TileContext(nc) as tc:
        with (
            tc.tile_pool(name="posp", bufs=1) as posp,
            tc.tile_pool(name="ancp", bufs=NBL) as ancp,
            tc.tile_pool(name="outp", bufs=1) as outp,
            tc.tile_pool(name="psmm", bufs=8, space="PSUM") as psmm,
        ):
            pos_t = posp.tile([P, MB, KT, P], F8)
            split = set(mode["hwdge"])
            anc_lo, anc_hi = [], []
            for b in range(NBL):
                w = widths[b]
                if b in split:
                    lo = ancp.tile([P, KT2, w], F8, tag=f"anclo{w}",
                                   name=f"anc{b}lo")
                    hi = ancp.tile([P, KT2, w], F8, tag=f"anchi{w}",
                                   name=f"anc{b}hi")
                else:
                    lo = ancp.tile([P, KT, w], F8, tag=f"ancfull{w}",
                                   name=f"anc{b}")
                    hi = lo
                anc_lo.append(lo)
                anc_hi.append(hi)

            def lo_dma(q, b):
                o = KT * offs[b]
                q.dma_start(anc_lo[b][:], ancTI[:, o:o + KT2 * widths[b]])

            def hi_dma(q, b):
                o = KT * offs[b] + KT2 * widths[b]
                q.dma_start(anc_hi[b][:], ancTI[:, o:o + KT2 * widths[b]])

            first = mode["hwdge"][0]
            lo_dma(nc.sync, first)
            hi_dma(nc.scalar, first)
            for mq in range(MB):
                nc.gpsimd.dma_start(pos_t[:, mq], posTI[:, mq, :])
            for b in mode["hwdge"][1:]:
                lo_dma(nc.sync, b)
                hi_dma(nc.scalar, b)
            for b in mode["swdge"]:
                o = KT * offs[b]
                nc.gpsimd.dma_start(anc_lo[b][:],
                                    ancTI[:, o:o + KT * widths[b]])

            rm_sb = outp.tile([P, NBL * MB], F32)

            # PE p-state warmup on a zeroed scratch tile
            warm_in = posp.tile([P, 2, NBLK], F8)
            nc.vector.memset(warm_in[:], 0)
            warm_ps = psmm.tile([P, NBLK], F32, tag="ps")
            for _ in range(warmups):
                nc.tensor.matmul(warm_ps[:], warm_in[:, :, 0:P],
                                 warm_in[:], start=True, stop=True,
                                 perf_mode=PM)

            for b in range(NBL):
                w = widths[b]
                for m in range(MB):
                    ps = psmm.tile([P, NBLK], F32, tag="ps")
                    for k in range(KT2):
                        if b in split:
                            src, kk = ((anc_lo[b], k) if k < 2
                                       else (anc_hi[b], k - 2))
                        else:
                            src, kk = anc_lo[b], k
                        nc.tensor.matmul(
                            ps[:, 0:w],
                            pos_t[:, m, 2 * k:2 * k + 2, :],
                            src[:, 2 * kk:2 * kk + 2, :],
                            start=(k == 0), stop=(k == KT2 - 1),
                            perf_mode=PM)
                    col = b * MB + m
                    nc.vector.tensor_reduce(
                        rm_sb[:, col:col + 1], ps[:, 0:w], AX.X, OP.max)
            nc.sync.dma_start(rmf[:], rm_sb[:])
    nc.compile()
    return nc


def _get_nc(**kw):
    key = ("nc",) + tuple(sorted(kw.items()))
    if key not in _CACHE:
        _CACHE[key] = _build(**kw)
    return _CACHE[key]


def _prep(x, widths):
    """Normalize in fp64, quantize to fp8, build per-core SBUF-layouts."""
    x = np.asarray(x, dtype=np.float32)
    assert x.shape == (B, M, D)
    pos = x[:, 0, :]
    anc = x[:, 1:, :].mean(axis=1) if M > 2 else x[:, 1, :]
    pos64 = pos.astype(np.float64)
    anc64 = anc.astype(np.float64)
    pn = np.maximum(np.linalg.norm(pos64, axis=1, keepdims=True), 1e-30)
    an = np.maximum(np.linalg.norm(anc64, axis=1, keepdims=True), 1e-30)
    posn64 = pos64 / pn
    ancn64 = anc64 / an

    pos8 = posn64.astype(np.float32).astype(NP_F8)   # [B, D]
    anc8 = ancn64.astype(np.float32).astype(NP_F8)

    # block b packed as [KT, w] row-major per partition:
    # chunk[p, KT*off.. ] with [k, c] c-inner
    a8T = anc8.T                                      # [D, B]
    chunks = []
    c0 = 0
    for w in widths:
        chunks.append(np.ascontiguousarray(
            a8T[:, c0:c0 + w].reshape(KT, P, w).transpose(1, 0, 2)
            .reshape(P, KT * w)))
        c0 += w
    ancTI = np.ascontiguousarray(np.concatenate(chunks, axis=1))

    in_maps = []
    for c in range(NCORES):
        sl = slice(c * RB, (c + 1) * RB)
        # posTI[p, m, k*128 + r] = posn[c*512 + m*128 + r][k*128 + p]
        posTI = np.ascontiguousarray(
            pos8[sl].T.reshape(KT, P, MB, P).transpose(1, 2, 0, 3)
            .reshape(P, MB, KT * P))
        in_maps.append({"posTI": posTI, "ancTI": ancTI})
    return in_maps, posn64, ancn64


def _run_cores(x, trace=False, **kw):
    in_maps, posn64, ancn64 = _prep(x, _widths(kw.get("dma_mode")))
    nc = _get_nc(**kw)
    res = run_bass_kernel_spmd(nc, in_maps, list(range(NCORES)), trace=trace)
    return res, posn64, ancn64


def _assemble(res, posn64, ancn64):
    # device row maxes: row i = 512c + 128m + p -> max over col-blocks
    rm = np.empty(B, np.float32)
    for c in range(NCORES):
        r = res.results[c]["rmf"]                     # [P, NBL*MB]
        r = r.reshape(P, r.shape[1] // MB, MB)        # [P, b, m]
        for m in range(MB):
            rows = slice(c * RB + m * P, c * RB + (m + 1) * P)
            rm[rows] = r[:, :, m].max(axis=1)

    diag = np.einsum("id,id->i", posn64, ancn64)      # exact fp64

    match = np.zeros(B, dtype=bool)
    suspect = diag >= (rm.astype(np.float64) - THRESH)
    if suspect.any():
        idx = np.where(suspect)[0]
        rows = posn64[idx] @ ancn64.T                 # exact fp64 rows
        match[idx] = rows.argmax(axis=1) == idx
    prec1 = np.float32(match.sum() / B * 100.0)

    # forward-pass flatNCE identity: exp(t - stop_grad(t)) == 1 per column
    nloss = np.float32(1.0)
    return nloss, prec1, rm


def kernel(x):
    res, posn64, ancn64 = _run_cores(x, trace=False)
    nloss, prec1, _ = _assemble(res, posn64, ancn64)
    return nloss, prec1
